# revision 1
# baseline (speedup 1.0000x reference)
# Trainium2 Bass kernel for nn_CN_MLP_71631464563230 (moe_routing).
#
# Math (classifier folded into the mixture matrices):
#   mlp_out = x @ W.T + b                        [B, H]
#   V[t,h]  = sum_k CM[t,h,k] * cla_w[k]         [T, H]
#   R       = [attn | V.T]                       [H, 2T]
#   aT, sT  = attn.T @ mlp_outT, V @ mlp_outT    [T, Bloc] each
#   out[b]  = sigmoid( (sum_t e^{a_tb} s_tb) / (sum_t e^{a_tb}) + cla_b )
#
# The mlp bias never materializes: asT = R.T @ raw_moT + (R.T b) 1^T, so it
# enters as per-partition constants c = R.T b in the epilogue (exp bias / s
# offset). The sigmoid is computed as exp+reciprocal so only the Exp ACT
# table is ever loaded (no table thrash on the critical tail).
#
# Sharding: batch 8x data-parallel (1024 rows/core); params replicated (a
# 4KB AllGather measures ~80us standalone here, so sharding CM+gathering V
# loses badly to replication). x/w in fp8e4 (w*64 host-side to clear the
# e4m3 denormal floor, inverted in the PSUM readout scale); main matmul
# runs DoubleRow (2 k-tiles per MM, ~1.7x bf16 rate).
#
# DMA reality: one ~420GB/s engine per core serves all queues, and every
# transfer pays a fixed setup cost, so bulk moves in few large transfers.
# The stream consumes x/w at only ~192GB/s average; x/w tiles are ordered
# by deadline with CM chunks filling the slack. CM stays bf16 — fp8 halves
# DVE throughput, costing more than the bytes saved. The V fold runs
# mid-stream on the DVE (mul at 2x, then two 2x half-adds + a small
# reduce, ~0.4us/row); ACT takes two chunks' sums via accum-copies
# (~1us/row — its accumulate needs an extra READ_ACCUMULATOR op, so only
# the overflow goes there). GPSIMD does no elementwise work: its ucode
# ops starve concurrent DVE reads ~7x and trigger multi-us library swaps.

import os

import ml_dtypes
import numpy as np

import concourse.bass as bass
import concourse.mybir as mybir
import concourse.tile as tile
from concourse import bacc
from concourse.bass_utils import run_bass_kernel_spmd

B, D, H, T = 8192, 5000, 512, 16
NCORES = 8
BLOC = B // NCORES            # 1024 batch rows per core
HALF = BLOC // 2              # 512
KT = (D + 127) // 128         # 40 k-tiles over D (last padded)
KP = KT // 2                  # 20 DoubleRow k-pairs
HT = H // 128                 # 4 h-tiles
XKT = [4, 4, 4, 4, 4, 4, 4, 4, 4, 4]  # k-tiles per x/w DMA tile
CMROWS = [8, 8, 8, 8, 8, 8, 8, 4, 2, 2]  # (t,j) rows per CM chunk
ACT_SUM = {0, 1, 2, 3}        # CM chunks whose sums run on ACT
N_WARM = 4                    # PE warm-up dummy matmuls
W_SCALE = 64.0

F32 = mybir.dt.float32
BF = mybir.dt.bfloat16
F8 = mybir.dt.float8e4
NP_BF = ml_dtypes.bfloat16
NP_F8 = mybir.dt.np(F8)       # ml_dtypes.float8_e4m3 (TRN semantics, max 240)
DR = mybir.MatmulPerfMode.DoubleRow
AF = mybir.ActivationFunctionType

LAST_RESULTS = None


def _build_nc():
    nc = bacc.Bacc("TRN2", target_bir_lowering=False)

    xT = nc.dram_tensor("xT", [128, KT * BLOC], F8, kind="ExternalInput").ap()
    wT = nc.dram_tensor("wT", [128, KT * H], F8, kind="ExternalInput").ap()
    cmb_d = nc.dram_tensor("cmb", [128, 64 * H], BF, kind="ExternalInput").ap()
    attnp = nc.dram_tensor("attnp", [128, HT * T], BF, kind="ExternalInput").ap()
    biasb = nc.dram_tensor("biasb", [128, HT], BF, kind="ExternalInput").ap()
    clar = nc.dram_tensor("clar", [128, H], BF, kind="ExternalInput").ap()
    clabn = nc.dram_tensor("clabn", [1, 1], F32, kind="ExternalInput").ap()
    out_d = nc.dram_tensor("out", [1, BLOC], F32, kind="ExternalOutput").ap()

    NCM = len(CMROWS)
    CMOFF = [sum(CMROWS[:c]) for c in range(NCM)]
    NXT = len(XKT)
    XOFF = [sum(XKT[:i]) for i in range(NXT)]
    kp2tile = []
    for i, n in enumerate(XKT):
        kp2tile += [i] * (n // 2)

    with tile.TileContext(nc) as tc:
        import contextlib

        ctx = contextlib.ExitStack()
        with ctx:
            sg = ctx.enter_context(tc.tile_pool(name="sg", bufs=1))
            pp = ctx.enter_context(tc.tile_pool(name="pp", bufs=1, space="PSUM"))

            # ---- tiles ----------------------------------------------------
            scrap = sg.tile([128, 2, HALF], F8, tag="scrap")
            ones16 = sg.tile([T, 1], BF, tag="ones16")
            warm1 = sg.tile([1, 1], F32, tag="warm1")
            R_sb = sg.tile([128, HT, 2 * T], BF, tag="R_sb")
            bias_sb = sg.tile([128, HT], BF, tag="bias_sb")
            clabn_sb = sg.tile([1, 1], F32, tag="clabn_sb")
            cla_rep = sg.tile([128, 8, H], BF, tag="cla_rep")
            cms = [sg.tile([128, CMROWS[c], H], BF, tag=f"cm{c}",
                           name=f"cm{c}") for c in range(NCM)]
            prods = [sg.tile([128, 8, H], BF, tag=f"prod{v % 3}",
                             name=f"prod{v}") for v in range(NCM)]
            padds = [sg.tile([128, 8, H // 2], BF, tag=f"padd{v % 3}",
                             name=f"padd{v}") for v in range(NCM)]
            padd2s = [sg.tile([128, 8, H // 4], BF, tag=f"padd2_{v % 2}",
                              name=f"padd2_{v}") for v in range(NCM)]

            ascr = sg.tile([128, H // 2], BF, tag="ascr")
            vt_all = sg.tile([128, T * HT], F32, tag="vt_all")
            xts = [sg.tile([128, XKT[i], BLOC], F8, tag=f"xt{i}",
                           name=f"xt{i}") for i in range(NXT)]
            wts = [sg.tile([128, XKT[i], H], F8, tag=f"wt{i}",
                           name=f"wt{i}") for i in range(NXT)]
            mo = [sg.tile([128, BLOC], BF, tag=f"mo{j}", name=f"mo{j}")
                  for j in range(HT)]
            ca_sb = sg.tile([T, 1], F32, tag="ca_sb")
            cs_sb = sg.tile([T, 1], F32, tag="cs_sb")
            E_sb = sg.tile([T, 2, HALF], BF, tag="E_sb")
            P_sb = sg.tile([T, 2, HALF], BF, tag="P_sb")
            rden_sb = sg.tile([1, 2, HALF], F32, tag="rden_sb")
            lg_sb = sg.tile([1, 2, HALF], F32, tag="lg_sb")
            eneg_sb = sg.tile([1, 2, HALF], F32, tag="eneg_sb")
            ep1_sb = sg.tile([1, 2, HALF], F32, tag="ep1_sb")
            orow = sg.tile([1, 2, HALF], F32, tag="orow")

            mm_ps = [pp.tile([128, HALF], F32, tag=f"p{i}", name=f"mm{i}")
                     for i in range(8)]

            # ---- warm-up: PE busy from t~0 (zero operands, results are
            # overwritten by the start=True of the real accumulation) -------
            nc.vector.memset(scrap, 0)
            nc.gpsimd.memset(ones16, 1.0)
            for i in range(N_WARM):
                nc.tensor.matmul(mm_ps[0], lhsT=scrap[:, :, 0:128], rhs=scrap,
                                 start=True, stop=True, perf_mode=DR,
                                 skip_group_check=True)

            # ---- small DMAs; Exp table preload ----------------------------
            nc.scalar.dma_start(out=cla_rep[:, 0, :], in_=clar)
            nc.scalar.dma_start(
                out=R_sb[:, :, 0:T],
                in_=attnp.rearrange("p (j t) -> p j t", t=T))
            nc.scalar.dma_start(out=bias_sb, in_=biasb)
            nc.scalar.dma_start(out=clabn_sb, in_=clabn)
            nc.scalar.activation(warm1, clabn_sb, AF.Exp)

            # ---- bulk DMAs, one queue, x/w by deadline, CM in the slack ---
            def x_dma(i):
                lo = XOFF[i]
                n = XKT[i]
                nc.sync.dma_start(
                    out=xts[i].rearrange("p c b -> p (c b)"),
                    in_=xT[:, lo * BLOC:(lo + n) * BLOC])

            def w_dma(i):
                lo = XOFF[i]
                n = XKT[i]
                nc.sync.dma_start(
                    out=wts[i].rearrange("p c h -> p (c h)"),
                    in_=wT[:, lo * H:(lo + n) * H])

            def cm_dma(c):
                nc.sync.dma_start(
                    out=cms[c].rearrange("p s k -> p (s k)"),
                    in_=cmb_d[:, CMOFF[c] * H:(CMOFF[c] + CMROWS[c]) * H])

            for item in ["x0", "w0", "x1", "w1", "c0", "x2", "w2", "c1",
                         "x3", "w3", "c2", "c3", "x4", "w4", "c4", "c5",
                         "x5", "w5", "x6", "w6", "c6", "x7", "w7", "c7",
                         "x8", "w8", "x9", "w9", "c8", "c9"]:
                kind, idx = item[0], int(item[1:])
                (x_dma if kind == "x" else w_dma if kind == "w"
                 else cm_dma)(idx)

            # ---- V fold, chunk-pipelined on DVE: mul (2x), half-add (2x),
            # quarter-add (2x), small reduce; two chunks' sums on ACT -------
            for s in range(1, 8):
                nc.vector.tensor_copy(cla_rep[:, s, :], cla_rep[:, 0, :])
            for c in range(NCM):
                r = CMROWS[c]
                nc.vector.tensor_mul(prods[c][:, 0:r, :], cms[c],
                                     cla_rep[:, 0:r, :])
                nc.vector.tensor_add(padds[c][:, 0:r, :],
                                     prods[c][:, 0:r, 0:H // 2],
                                     prods[c][:, 0:r, H // 2:H])
                vslice = vt_all[:, CMOFF[c]:CMOFF[c] + r]
                if c in ACT_SUM:
                    for s in range(r):
                        nc.scalar.activation(
                            ascr, padds[c][:, s, :], AF.Copy,
                            accum_out=vslice[:, s:s + 1])
                else:
                    nc.vector.tensor_add(padd2s[c][:, 0:r, :],
                                         padds[c][:, 0:r, 0:H // 4],
                                         padds[c][:, 0:r, H // 4:H // 2])
                    nc.vector.reduce_sum(vslice, padd2s[c][:, 0:r, :],
                                         axis=mybir.AxisListType.X)
            # R V-columns: R[:, j, T+t] = vt_all[:, t*HT+j]
            vt_v = vt_all.rearrange("p (t j) -> p t j", j=HT)
            for j in range(HT):
                nc.vector.tensor_copy(R_sb[:, j, T:2 * T], vt_v[:, :, j])

            # ---- main matmul: fp8 DoubleRow, all 8 PSUM banks -------------
            for kp in range(KP):
                ti = kp2tile[kp]
                xt_t = xts[ti]
                wt_t = wts[ti]
                lk = (kp - XOFF[ti] // 2) * 2
                for j in range(HT):
                    lhsT = wt_t[:, lk:lk + 2, j * 128:(j + 1) * 128]
                    for half in range(2):
                        nc.tensor.matmul(
                            mm_ps[j * 2 + half],
                            lhsT=lhsT,
                            rhs=xt_t[:, lk:lk + 2,
                                     half * HALF:(half + 1) * HALF],
                            start=(kp == 0), stop=(kp == KP - 1),
                            perf_mode=DR)

            # ---- epilogue -------------------------------------------------
            # mo[j] = raw mlp_outT / W_SCALE (bf16); emitted in bank order so
            # each copy chases its bank's last accumulating matmul
            for j in range(HT):
                for half in range(2):
                    dst = mo[j][:, half * HALF:(half + 1) * HALF]
                    if half == 0:
                        nc.vector.tensor_scalar_mul(dst, mm_ps[j * 2 + half],
                                                    1.0 / W_SCALE)
                    else:
                        nc.scalar.activation(dst, mm_ps[j * 2 + half], AF.Copy,
                                             scale=1.0 / W_SCALE)

            # c = R.T b  (per-partition epilogue constants)
            ca_ps = pp.tile([T, 1], F32, tag="p0", name="ca_ps")
            cs_ps = pp.tile([T, 1], F32, tag="p1", name="cs_ps")
            for j in range(HT):
                nc.tensor.matmul(ca_ps, lhsT=R_sb[:, j, 0:T],
                                 rhs=bias_sb[:, j:j + 1],
                                 start=(j == 0), stop=(j == HT - 1))
            for j in range(HT):
                nc.tensor.matmul(cs_ps, lhsT=R_sb[:, j, T:2 * T],
                                 rhs=bias_sb[:, j:j + 1],
                                 start=(j == 0), stop=(j == HT - 1))
            nc.vector.tensor_copy(ca_sb, ca_ps)
            nc.vector.tensor_copy(cs_sb, cs_ps)

            # asT per half: a rows (attn cols, ready early) then s rows (V)
            for half in range(2):
                a_ps = pp.tile([T, HALF], F32, tag=f"p{2 + 2 * half}",
                               name=f"a_ps{half}")
                s_ps = pp.tile([T, HALF], F32, tag=f"p{3 + 2 * half}",
                               name=f"s_ps{half}")
                for j in range(HT):
                    nc.tensor.matmul(
                        a_ps, lhsT=R_sb[:, j, 0:T],
                        rhs=mo[j][:, half * HALF:(half + 1) * HALF],
                        start=(j == 0), stop=(j == HT - 1))
                for j in range(HT):
                    nc.tensor.matmul(
                        s_ps, lhsT=R_sb[:, j, T:2 * T],
                        rhs=mo[j][:, half * HALF:(half + 1) * HALF],
                        start=(j == 0), stop=(j == HT - 1))
                nc.scalar.activation(E_sb[:, half, :], a_ps, AF.Exp,
                                     bias=ca_sb)
                # P = (sT + cs) * E in one fused DVE op
                nc.vector.scalar_tensor_tensor(
                    out=P_sb[:, half, :], in0=s_ps, scalar=cs_sb,
                    in1=E_sb[:, half, :], op0=mybir.AluOpType.add,
                    op1=mybir.AluOpType.mult)

            # softmax-combine: den/num via 16-row sum matmuls, then the
            # final chain per half so the two halves pipeline;
            # sigmoid(z + cla_b) = 1 / (1 + exp(-z - cla_b)), Exp table only
            dn_tags = [("p6", "p7"), ("p0", "p1")]
            for half in range(2):
                den_ps = pp.tile([1, HALF], F32, tag=dn_tags[half][0],
                                 name=f"den{half}")
                num_ps = pp.tile([1, HALF], F32, tag=dn_tags[half][1],
                                 name=f"num{half}")
                nc.tensor.matmul(den_ps, lhsT=ones16,
                                 rhs=E_sb[:, half, :], start=True, stop=True)
                nc.tensor.matmul(num_ps, lhsT=ones16,
                                 rhs=P_sb[:, half, :], start=True, stop=True)
                nc.vector.reciprocal_approx_fast(
                    out=rden_sb[:, half, :], in_=den_ps)
                nc.vector.tensor_mul(lg_sb[:, half, :], num_ps,
                                     rden_sb[:, half, :])
                nc.scalar.activation(eneg_sb[:, half, :], lg_sb[:, half, :],
                                     AF.Exp, bias=clabn_sb, scale=-1.0)
                nc.vector.tensor_scalar_add(ep1_sb[:, half, :],
                                            eneg_sb[:, half, :], 1.0)
                nc.vector.reciprocal_approx_fast(
                    out=orow[:, half, :], in_=ep1_sb[:, half, :])
            nc.sync.dma_start(out=out_d, in_=orow.rearrange("p a b -> p (a b)"))

    nc.finalize()
    return nc


_NC_CACHE = None


def _pack_inputs(data_input, mlp_w, mlp_b, CM, attn, cla_w, cla_b):
    x = np.ascontiguousarray(np.asarray(data_input, dtype=np.float32))
    w = np.asarray(mlp_w, dtype=np.float32)
    b = np.asarray(mlp_b, dtype=np.float32)
    CM = np.asarray(CM, dtype=np.float32)
    attn = np.asarray(attn, dtype=np.float32)
    cla_w = np.asarray(cla_w, dtype=np.float32).reshape(H)
    cla_b = np.asarray(cla_b, dtype=np.float32).reshape(1, 1)

    DP = KT * 128
    # x: [B, D] -> per core [128, KT*BLOC] fp8 (partition = k%128)
    xp = np.zeros((B, DP), dtype=np.float32)
    xp[:, :D] = np.clip(x, -240, 240)
    xp = (xp.reshape(NCORES, BLOC, KT, 128)
            .transpose(0, 3, 2, 1)          # [core, 128, KT, BLOC]
            .reshape(NCORES, 128, KT * BLOC)
            .astype(NP_F8))
    # w: [H, D] -> [128, KT*H] fp8, scaled
    wp = np.zeros((DP, H), dtype=np.float32)
    wp[:D, :] = np.clip(w.T * W_SCALE, -240, 240)
    wp = (wp.reshape(KT, 128, H).transpose(1, 0, 2)
            .reshape(128, KT * H).astype(NP_F8))
    # CM replicated: [128, (t j) k] bf16
    cmb = (CM.reshape(T, HT, 128, H)
             .transpose(2, 0, 1, 3)         # [128, T, HT, H]
             .reshape(128, 64 * H).astype(NP_BF))
    attnp = (attn.reshape(HT, 128, T).transpose(1, 0, 2)
                 .reshape(128, HT * T).astype(NP_BF))
    biasb = np.ascontiguousarray(b.reshape(HT, 128).T.astype(NP_BF))
    clar = np.ascontiguousarray(
        np.broadcast_to(cla_w.astype(NP_BF), (128, H)))
    clabn = np.ascontiguousarray(-cla_b)

    shared = {"wT": wp, "cmb": cmb, "attnp": attnp,
              "biasb": biasb, "clar": clar, "clabn": clabn}
    return [
        {"xT": np.ascontiguousarray(xp[i]), **shared}
        for i in range(NCORES)
    ]


def kernel(data_input, mlp_w, mlp_b, CM, attn, cla_w, cla_b):
    global LAST_RESULTS, _NC_CACHE

    in_maps = _pack_inputs(data_input, mlp_w, mlp_b, CM, attn, cla_w, cla_b)

    if _NC_CACHE is None:
        _NC_CACHE = _build_nc()

    trace = bool(int(os.environ.get("KERNEL_TRACE", "0")))
    res = run_bass_kernel_spmd(
        _NC_CACHE, in_maps, core_ids=list(range(NCORES)), trace=trace,
        trace_cores=[0] if trace else None,
    )
    LAST_RESULTS = res

    full = np.empty(B, dtype=np.float32)
    for i in range(NCORES):
        full[i * BLOC:(i + 1) * BLOC] = res.results[i]["out"].reshape(BLOC)
    return full



# revision 3
# speedup vs baseline: 2.1173x; 2.1173x over previous
# Trainium2 Bass kernel for nn_CN_MLP_71631464563230 (moe_routing).
#
# Math: the reference is
#   mo = x @ W.T + b;  w = softmax(mo @ attn);  out = sigmoid(w . (mo @ V.T) + cla_b)
# with V[t,h] = sum_k CM[t,h,k] cla_w[k]. Both pre-softmax quantities are
# LINEAR in mo, and mo is affine in x, so by associativity
#   a  = mo @ attn = x @ (W.T @ attn) + (b @ attn)
#   s  = mo @ V.T  = x @ (W.T @ V.T)  + (V @ b)
# The parameter-only folds G = [W.T attn | W.T V.T] (D x 2T), ca, cs are
# precomputed on the host at pack time (constant folding through linear
# layers, like BN-into-conv). The device computes asT = G.T x.T per core
# (fp8 DoubleRow, 16x fewer MACs than materializing mo) and the nonlinear
# epilogue:  out = sigmoid( (sum_t e^{a} s)/(sum_t e^{a}) + cla_b ).
# CM / mlp_w / attn never need to be DMA'd: per-core traffic drops from
# 16.4MB to 5.4MB, and the ~40us DVE V-fold of the previous version
# disappears entirely.
#
# Sharding: batch 8x data-parallel (1024 rows/core); G + consts replicated
# (G is 160KB fp8). G is scaled by 64 host-side to clear the e4m3 denormal
# floor (G elems ~ N(0, 1/5000)); the scale is inverted in the ACT scale
# operand of the two Exp ops, and cs is pre-multiplied by it, so no extra
# device ops. The sigmoid is exp+reciprocal so only the Exp ACT table is
# ever loaded.
#
# The batch is split into 4 column groups of 256 so each group's epilogue
# (Exp / (s+cs)*E / 16-row-sum matmuls / reciprocal chain) overlaps the
# next group's DMA+matmul stream; only the last group's chain is an
# exposed tail. x streams group-major in 8-ktile chunks (2KB/partition
# per transfer) on one queue; the kernel is x-DMA-bound (~5.4MB/core).

import os

import ml_dtypes
import numpy as np

import concourse.bass as bass
import concourse.mybir as mybir
import concourse.tile as tile
from concourse import bacc
from concourse.bass_utils import run_bass_kernel_spmd

B, D, H, T = 8192, 5000, 512, 16
NCORES = 8
BLOC = B // NCORES            # 1024 batch rows per core
KT = (D + 127) // 128         # 40 k-tiles over D (last padded)
KP = KT // 2                  # 20 DoubleRow k-pairs
NG = 4                        # batch column groups per core
GB = BLOC // NG               # 256 batch rows per group
CKT = 8                       # k-tiles per x DMA chunk
NCH = KT // CKT               # 5 chunks per group
G_SCALE = 64.0
M2 = 64                       # PE out partitions: a at 0:16, s at 32:48 (quadrant-aligned)

F32 = mybir.dt.float32
BF = mybir.dt.bfloat16
F8 = mybir.dt.float8e4
NP_F8 = mybir.dt.np(F8)       # ml_dtypes.float8_e4m3 (TRN semantics, max 240)
DR = mybir.MatmulPerfMode.DoubleRow
AF = mybir.ActivationFunctionType

LAST_RESULTS = None


def _build_nc():
    nc = bacc.Bacc("TRN2", target_bir_lowering=False)

    xT = nc.dram_tensor("xT", [128, NG * KT * GB], F8, kind="ExternalInput").ap()
    gT = nc.dram_tensor("gT", [128, KT * M2], F8, kind="ExternalInput").ap()
    ca_d = nc.dram_tensor("ca", [T, 1], F32, kind="ExternalInput").ap()
    cs_d = nc.dram_tensor("csp", [T, 1], F32, kind="ExternalInput").ap()
    clabn = nc.dram_tensor("clabn", [1, 1], F32, kind="ExternalInput").ap()
    out_d = nc.dram_tensor("out", [1, BLOC], F32, kind="ExternalOutput").ap()

    with tile.TileContext(nc) as tc:
        import contextlib

        ctx = contextlib.ExitStack()
        with ctx:
            sg = ctx.enter_context(tc.tile_pool(name="sg", bufs=1))
            pp = ctx.enter_context(tc.tile_pool(name="pp", bufs=1, space="PSUM"))

            # ---- tiles ----------------------------------------------------
            G_sb = sg.tile([128, KT, M2], F8, tag="G_sb")
            xch = [sg.tile([128, CKT, GB], F8, tag=f"x{g}_{c}",
                           name=f"x{g}_{c}")
                   for g in range(NG) for c in range(NCH)]
            ca_sb = sg.tile([T, 1], F32, tag="ca_sb")
            cs_sb = sg.tile([T, 1], F32, tag="cs_sb")
            clabn_sb = sg.tile([1, 1], F32, tag="clabn_sb")
            ones16 = sg.tile([T, 1], BF, tag="ones16")
            warm1 = sg.tile([1, 1], F32, tag="warm1")
            E_sb = sg.tile([T, NG, GB], BF, tag="E_sb")
            P_sb = sg.tile([T, NG, GB], BF, tag="P_sb")
            rden = sg.tile([1, NG, GB], F32, tag="rden")
            lg = sg.tile([1, NG, GB], F32, tag="lg")
            eneg = sg.tile([1, NG, GB], F32, tag="eneg")
            ep1 = sg.tile([1, NG, GB], F32, tag="ep1")
            orow = sg.tile([1, NG, GB], F32, tag="orow")

            mm_ps = [pp.tile([M2, GB], F32, tag=f"p{g}", name=f"mm{g}")
                     for g in range(NG)]

            # ---- DMA triggers first: x stream starts ASAP -----------------
            nc.sync.dma_start(
                out=G_sb.rearrange("p k m -> p (k m)"), in_=gT)
            for g in range(NG):
                for c in range(NCH):
                    lo = (g * KT + c * CKT) * GB
                    nc.sync.dma_start(
                        out=xch[g * NCH + c].rearrange("p k b -> p (k b)"),
                        in_=xT[:, lo:lo + CKT * GB])

            # ---- small consts; Exp table preload --------------------------
            nc.scalar.dma_start(out=ca_sb, in_=ca_d)
            nc.scalar.dma_start(out=cs_sb, in_=cs_d)
            nc.scalar.dma_start(out=clabn_sb, in_=clabn)
            nc.gpsimd.memset(ones16, 1.0)
            nc.scalar.activation(warm1, clabn_sb, AF.Exp)

            # ---- per-group: 20 DR matmuls, then epilogue ------------------
            def epilogue(g):
                # E = exp(a/G_SCALE + ca);  P = (s + G_SCALE*cs) * E
                nc.scalar.activation(E_sb[:, g, :], mm_ps[g][0:T, :], AF.Exp,
                                     bias=ca_sb, scale=1.0 / G_SCALE)
                nc.vector.scalar_tensor_tensor(
                    out=P_sb[:, g, :], in0=mm_ps[g][32:48, :], scalar=cs_sb,
                    in1=E_sb[:, g, :], op0=mybir.AluOpType.add,
                    op1=mybir.AluOpType.mult)
                dn = 4 + 2 * (g % 2)
                den_ps = pp.tile([1, GB], F32, tag=f"p{dn}", name=f"den{g}")
                num_ps = pp.tile([1, GB], F32, tag=f"p{dn + 1}",
                                 name=f"num{g}")
                nc.tensor.matmul(den_ps, lhsT=ones16, rhs=E_sb[:, g, :],
                                 start=True, stop=True)
                nc.tensor.matmul(num_ps, lhsT=ones16, rhs=P_sb[:, g, :],
                                 start=True, stop=True)
                # sigmoid(num/den/G_SCALE + cla_b) via Exp + fast reciprocal
                nc.vector.reciprocal_approx_fast(
                    out=rden[:, g, :], in_=den_ps)
                nc.vector.tensor_mul(lg[:, g, :], num_ps, rden[:, g, :])
                nc.scalar.activation(eneg[:, g, :], lg[:, g, :], AF.Exp,
                                     bias=clabn_sb, scale=-1.0 / G_SCALE)
                nc.vector.tensor_scalar_add(ep1[:, g, :], eneg[:, g, :], 1.0)
                nc.vector.reciprocal_approx_fast(
                    out=orow[:, g, :], in_=ep1[:, g, :])

            for g in range(NG):
                for c in range(NCH):
                    xt = xch[g * NCH + c]
                    for i in range(CKT // 2):
                        kp = c * (CKT // 2) + i
                        nc.tensor.matmul(
                            mm_ps[g],
                            lhsT=G_sb[:, 2 * kp:2 * kp + 2, :],
                            rhs=xt[:, 2 * i:2 * i + 2, :],
                            start=(kp == 0), stop=(kp == KP - 1),
                            perf_mode=DR)
                # epilogue of the previous group lands while this group's
                # stream continues; PE sees its den/num matmuls only after
                # the next group's main matmuls, so it never waits on ACT
                if g >= 1:
                    epilogue(g - 1)
            epilogue(NG - 1)

            nc.sync.dma_start(out=out_d,
                              in_=orow.rearrange("p g b -> p (g b)"))

    nc.finalize()
    return nc


_NC_CACHE = None


def _pack_inputs(data_input, mlp_w, mlp_b, CM, attn, cla_w, cla_b):
    x = np.asarray(data_input, dtype=np.float32)
    W = np.asarray(mlp_w, dtype=np.float32)
    b = np.asarray(mlp_b, dtype=np.float32)
    CM = np.asarray(CM, dtype=np.float32)
    attn = np.asarray(attn, dtype=np.float32)
    cla_w = np.asarray(cla_w, dtype=np.float32).reshape(H)
    cla_b = np.asarray(cla_b, dtype=np.float32).reshape(1, 1)

    # Parameter folds (host, O(D*H) — data-independent)
    V = CM @ cla_w                       # [T, H]
    Ga = W.T @ attn                      # [D, T]
    Gs = W.T @ V.T                       # [D, T]
    ca = (b @ attn).reshape(T, 1)
    csp = (G_SCALE * (V @ b)).reshape(T, 1)

    DP = KT * 128
    # x: [B, D] -> per core [128, (g kt j)] fp8, group-major
    xp = np.zeros((B, DP), dtype=np.float32)
    xp[:, :D] = np.clip(x, -240, 240)
    xp = (xp.reshape(NCORES, NG, GB, KT, 128)
            .transpose(0, 4, 1, 3, 2)        # [core, 128, g, kt, j]
            .reshape(NCORES, 128, NG * KT * GB)
            .astype(NP_F8))
    # G: [D, 2T] -> [128, (kt m)] fp8, scaled
    gp = np.zeros((DP, M2), dtype=np.float32)
    gp[:D, 0:T] = np.clip(Ga * G_SCALE, -240, 240)
    gp[:D, 32:32 + T] = np.clip(Gs * G_SCALE, -240, 240)
    gp = (gp.reshape(KT, 128, M2).transpose(1, 0, 2)
            .reshape(128, KT * M2).astype(NP_F8))

    shared = {"gT": gp, "ca": np.ascontiguousarray(ca),
              "csp": np.ascontiguousarray(csp),
              "clabn": np.ascontiguousarray(-cla_b)}
    return [
        {"xT": np.ascontiguousarray(xp[i]), **shared}
        for i in range(NCORES)
    ]


def kernel(data_input, mlp_w, mlp_b, CM, attn, cla_w, cla_b):
    global LAST_RESULTS, _NC_CACHE

    in_maps = _pack_inputs(data_input, mlp_w, mlp_b, CM, attn, cla_w, cla_b)

    if _NC_CACHE is None:
        _NC_CACHE = _build_nc()

    trace = bool(int(os.environ.get("KERNEL_TRACE", "0")))
    res = run_bass_kernel_spmd(
        _NC_CACHE, in_maps, core_ids=list(range(NCORES)), trace=trace,
        trace_cores=[0] if trace else None,
    )
    LAST_RESULTS = res

    full = np.empty(B, dtype=np.float32)
    for i in range(NCORES):
        full[i * BLOC:(i + 1) * BLOC] = res.results[i]["out"].reshape(BLOC)
    return full


# revision 4
# speedup vs baseline: 2.1348x; 1.0083x over previous
# Trainium2 Bass kernel for nn_CN_MLP_71631464563230 (moe_routing).
#
# Math: the reference is
#   mo = x @ W.T + b;  w = softmax(mo @ attn);  out = sigmoid(w . (mo @ V.T) + cla_b)
# with V[t,h] = sum_k CM[t,h,k] cla_w[k]. Both pre-softmax quantities are
# LINEAR in mo, and mo is affine in x, so by associativity
#   a  = mo @ attn = x @ (W.T @ attn) + (b @ attn)
#   s  = mo @ V.T  = x @ (W.T @ V.T)  + (V @ b)
# The parameter-only folds G = [W.T attn | W.T V.T] (D x 2T), ca, cs are
# precomputed on the host at pack time (constant folding through linear
# layers, like BN-into-conv). The device computes asT = G.T x.T per core
# (fp8 DoubleRow, 16x fewer MACs than materializing mo) and the nonlinear
# epilogue:  out = sigmoid( (sum_t e^{a} s)/(sum_t e^{a}) + cla_b ).
# CM / mlp_w / attn never need to be DMA'd: per-core traffic drops from
# 16.4MB to 5.4MB, and the old ~40us DVE V-fold disappears entirely.
#
# Sharding: batch 8x data-parallel (1024 rows/core); G + consts replicated
# (G is 320KB fp8). G is scaled by 64 host-side to clear the e4m3 denormal
# floor (G elems ~ N(0, 1/5000)); the scale is inverted in the ACT scale
# operand of the two Exp ops, and cs is pre-multiplied by it. a lands in
# PSUM partitions 0:16 and s in 32:48 (DVE reads of PSUM must be
# 32-partition-quadrant aligned). Only the Exp ACT table is ever loaded.
#
# DMA reality: each dma_start costs ~700ns of trigger time on its issuing
# engine and partition lines under ~4KB drop throughput, so x streams in 9
# big transfers (5KB lines) on the sync queue while G + consts go on the
# scalar queue in parallel. The batch is split into 4 groups of 256 so
# each group's epilogue overlaps the next group's stream; the last group
# tapers (20/14/6 k-tiles) so little matmul work trails the final byte.
# E and P share one tile so den|num is a single 16-row-sum matmul; each
# group DMAs its own output slice out as soon as it is ready.

import os

import ml_dtypes
import numpy as np

import concourse.bass as bass
import concourse.mybir as mybir
import concourse.tile as tile
from concourse import bacc
from concourse.bass_utils import run_bass_kernel_spmd

B, D, H, T = 8192, 5000, 512, 16
NCORES = 8
BLOC = B // NCORES            # 1024 batch rows per core
KT = (D + 127) // 128         # 40 k-tiles over D (last padded)
KP = KT // 2                  # 20 DoubleRow k-pairs
NG = 4                        # batch column groups per core
GB = BLOC // NG               # 256 batch rows per group
CHUNKS = [[(0, 20), (20, 40)]] * 3 + [[(0, 20), (20, 34), (34, 40)]]
G_SCALE = 64.0
M2 = 64                       # PE out partitions: a at 0:16, s at 32:48

F32 = mybir.dt.float32
BF = mybir.dt.bfloat16
F8 = mybir.dt.float8e4
NP_F8 = mybir.dt.np(F8)       # ml_dtypes.float8_e4m3 (TRN semantics, max 240)
DR = mybir.MatmulPerfMode.DoubleRow
AF = mybir.ActivationFunctionType

LAST_RESULTS = None


def _build_nc():
    nc = bacc.Bacc("TRN2", target_bir_lowering=False)

    xT = nc.dram_tensor("xT", [128, NG * KT * GB], F8, kind="ExternalInput").ap()
    gT = nc.dram_tensor("gT", [128, KT * M2], F8, kind="ExternalInput").ap()
    ca_d = nc.dram_tensor("ca", [T, 1], F32, kind="ExternalInput").ap()
    cs_d = nc.dram_tensor("csp", [T, 1], F32, kind="ExternalInput").ap()
    clabn = nc.dram_tensor("clabn", [1, 1], F32, kind="ExternalInput").ap()
    out_d = nc.dram_tensor("out", [1, BLOC], F32, kind="ExternalOutput").ap()

    with tile.TileContext(nc) as tc:
        import contextlib

        ctx = contextlib.ExitStack()
        with ctx:
            sg = ctx.enter_context(tc.tile_pool(name="sg", bufs=1))
            pp = ctx.enter_context(tc.tile_pool(name="pp", bufs=1, space="PSUM"))

            # ---- tiles ----------------------------------------------------
            G_sb = sg.tile([128, KT, M2], F8, tag="G_sb")
            xch = {}
            for g in range(NG):
                for (k0, k1) in CHUNKS[g]:
                    xch[g, k0] = sg.tile([128, k1 - k0, GB], F8,
                                         tag=f"x{g}_{k0}", name=f"x{g}_{k0}")
            ca_sb = sg.tile([T, 1], F32, tag="ca_sb")
            cs_sb = sg.tile([T, 1], F32, tag="cs_sb")
            clabn_sb = sg.tile([1, 1], F32, tag="clabn_sb")
            ones16 = sg.tile([T, 1], BF, tag="ones16")
            warm1 = sg.tile([1, 1], F32, tag="warm1")
            # E and P adjacent so den|num is one 16-row-sum matmul
            EP_sb = sg.tile([T, NG, 2, GB], BF, tag="EP_sb")
            rden = sg.tile([1, NG, GB], F32, tag="rden")
            lg = sg.tile([1, NG, GB], F32, tag="lg")
            eneg = sg.tile([1, NG, GB], F32, tag="eneg")
            ep1 = sg.tile([1, NG, GB], F32, tag="ep1")
            orow = sg.tile([1, NG, GB], F32, tag="orow")

            mm_ps = [pp.tile([M2, GB], F32, tag=f"p{g}", name=f"mm{g}")
                     for g in range(NG)]

            # ---- x stream on the sync queue, big transfers ----------------
            for g in range(NG):
                for (k0, k1) in CHUNKS[g]:
                    lo = (g * KT + k0) * GB
                    nc.sync.dma_start(
                        out=xch[g, k0].rearrange("p k b -> p (k b)"),
                        in_=xT[:, lo:lo + (k1 - k0) * GB])

            # ---- G + consts on the scalar queue; Exp table preload --------
            nc.scalar.dma_start(
                out=G_sb.rearrange("p k m -> p (k m)"), in_=gT)
            nc.scalar.dma_start(out=ca_sb, in_=ca_d)
            nc.scalar.dma_start(out=cs_sb, in_=cs_d)
            nc.scalar.dma_start(out=clabn_sb, in_=clabn)
            nc.gpsimd.memset(ones16, 1.0)
            nc.scalar.activation(warm1, clabn_sb, AF.Exp)

            # ---- per-group: 20 DR matmuls, then epilogue ------------------
            def epilogue(g):
                # E = exp(a/G_SCALE + ca);  P = (s + G_SCALE*cs) * E
                nc.scalar.activation(EP_sb[:, g, 0, :], mm_ps[g][0:T, :],
                                     AF.Exp, bias=ca_sb, scale=1.0 / G_SCALE)
                nc.vector.scalar_tensor_tensor(
                    out=EP_sb[:, g, 1, :], in0=mm_ps[g][32:48, :],
                    scalar=cs_sb, in1=EP_sb[:, g, 0, :],
                    op0=mybir.AluOpType.add, op1=mybir.AluOpType.mult)
                dn_ps = pp.tile([1, 2 * GB], F32, tag=f"p{4 + g % 2}",
                                name=f"dn{g}")
                nc.tensor.matmul(
                    dn_ps, lhsT=ones16,
                    rhs=EP_sb[:, g, :, :].rearrange("t a b -> t (a b)"),
                    start=True, stop=True)
                # sigmoid(num/den/G_SCALE + cla_b) via Exp + fast reciprocal
                nc.vector.reciprocal_approx_fast(
                    out=rden[:, g, :], in_=dn_ps[:, 0:GB])
                nc.vector.tensor_mul(lg[:, g, :], dn_ps[:, GB:2 * GB],
                                     rden[:, g, :])
                nc.scalar.activation(eneg[:, g, :], lg[:, g, :], AF.Exp,
                                     bias=clabn_sb, scale=-1.0 / G_SCALE)
                nc.vector.tensor_scalar_add(ep1[:, g, :], eneg[:, g, :], 1.0)
                nc.vector.reciprocal_approx_fast(
                    out=orow[:, g, :], in_=ep1[:, g, :])
                nc.sync.dma_start(out=out_d[:, g * GB:(g + 1) * GB],
                                  in_=orow[:, g, :])

            for g in range(NG):
                for (k0, k1) in CHUNKS[g]:
                    xt = xch[g, k0]
                    for kp in range(k0 // 2, k1 // 2):
                        lk = 2 * kp - k0
                        nc.tensor.matmul(
                            mm_ps[g],
                            lhsT=G_sb[:, 2 * kp:2 * kp + 2, :],
                            rhs=xt[:, lk:lk + 2, :],
                            start=(kp == 0), stop=(kp == KP - 1),
                            perf_mode=DR)
                # epilogue of the previous group lands while this group's
                # stream continues; PE sees its den|num matmul only after
                # the next group's main matmuls, so it never waits on ACT
                if g >= 1:
                    epilogue(g - 1)
            epilogue(NG - 1)

    nc.finalize()
    return nc


_NC_CACHE = None


def _pack_inputs(data_input, mlp_w, mlp_b, CM, attn, cla_w, cla_b):
    x = np.asarray(data_input, dtype=np.float32)
    W = np.asarray(mlp_w, dtype=np.float32)
    b = np.asarray(mlp_b, dtype=np.float32)
    CM = np.asarray(CM, dtype=np.float32)
    attn = np.asarray(attn, dtype=np.float32)
    cla_w = np.asarray(cla_w, dtype=np.float32).reshape(H)
    cla_b = np.asarray(cla_b, dtype=np.float32).reshape(1, 1)

    # Parameter folds (host, O(D*H) — data-independent)
    V = CM @ cla_w                       # [T, H]
    Ga = W.T @ attn                      # [D, T]
    Gs = W.T @ V.T                       # [D, T]
    ca = (b @ attn).reshape(T, 1)
    csp = (G_SCALE * (V @ b)).reshape(T, 1)

    DP = KT * 128
    # x: [B, D] -> per core [128, (g kt j)] fp8, group-major
    xp = np.zeros((B, DP), dtype=np.float32)
    xp[:, :D] = np.clip(x, -240, 240)
    xp = (xp.reshape(NCORES, NG, GB, KT, 128)
            .transpose(0, 4, 1, 3, 2)        # [core, 128, g, kt, j]
            .reshape(NCORES, 128, NG * KT * GB)
            .astype(NP_F8))
    # G: [D, 2T] -> [128, (kt m)] fp8, scaled, quadrant-padded
    gp = np.zeros((DP, M2), dtype=np.float32)
    gp[:D, 0:T] = np.clip(Ga * G_SCALE, -240, 240)
    gp[:D, 32:32 + T] = np.clip(Gs * G_SCALE, -240, 240)
    gp = (gp.reshape(KT, 128, M2).transpose(1, 0, 2)
            .reshape(128, KT * M2).astype(NP_F8))

    shared = {"gT": gp, "ca": np.ascontiguousarray(ca),
              "csp": np.ascontiguousarray(csp),
              "clabn": np.ascontiguousarray(-cla_b)}
    return [
        {"xT": np.ascontiguousarray(xp[i]), **shared}
        for i in range(NCORES)
    ]


def kernel(data_input, mlp_w, mlp_b, CM, attn, cla_w, cla_b):
    global LAST_RESULTS, _NC_CACHE

    in_maps = _pack_inputs(data_input, mlp_w, mlp_b, CM, attn, cla_w, cla_b)

    if _NC_CACHE is None:
        _NC_CACHE = _build_nc()

    trace = bool(int(os.environ.get("KERNEL_TRACE", "0")))
    res = run_bass_kernel_spmd(
        _NC_CACHE, in_maps, core_ids=list(range(NCORES)), trace=trace,
        trace_cores=[0] if trace else None,
    )
    LAST_RESULTS = res

    full = np.empty(B, dtype=np.float32)
    for i in range(NCORES):
        full[i * BLOC:(i + 1) * BLOC] = res.results[i]["out"].reshape(BLOC)
    return full
